# revision 23
# baseline (speedup 1.0000x reference)
"""Trainium2 Bass kernel for causal GQA self-attention with QK RMS-norm.

Problem (hardcoded): B=2, T=2048, d_model=2048, 16 Q heads / 4 KV heads,
head_dim=128, fp32 IO.

Sharding across 8 NeuronCores: tensor-parallel over the 4 KV head groups
(each group = 1 KV head + its 4 Q heads) x data-parallel over the 2
batches.  core = 4*b + g.  Each core computes
    qkvT_g = Wg.T @ x_b.T          ([768, T])
    q/k RMS-norm (+ per-dim scales), v transposed to natural layout
    causal attention for the 4 Q heads of group g (S^T orientation)
    yp_partial = (O^T).T @ Wp_g    ([T, d_model] partial)
and the host sums the 4 partials per batch.

fp8 scheme: W_qkv is pre-scaled by 16 on the host so its entries sit in
e4m3's normal range; q/k RMS-norm is scale-invariant (per-dim scales are
divided by 16 to compensate), and v's 16x is folded into W_proj/16 (bf16).
The QKV projection and the full-width (off-diagonal) PV + row-sum matmuls
run as fp8 DoubleRow (contraction pairs, 2x PE throughput).  exp carries a
-2 bias so P fits e4m3 (ratio O/rowsum is bias-invariant).  Diagonal
blocks and the output projection keep the bf16 path.
"""

import functools

import numpy as np

import concourse.bass as bass
import concourse.mybir as mybir
import concourse.tile as tile
from concourse import bacc
from concourse.bass_utils import run_bass_kernel_spmd
from concourse.masks import make_identity

F32 = mybir.dt.float32
BF16 = mybir.dt.bfloat16
F8 = mybir.dt.float8e4
DR = mybir.MatmulPerfMode.DoubleRow

MM_DT = BF16

T = 2048
C = 2048
D = 128
NH = 4            # q heads per core
NKC = C // 128    # 16 k-chunks of the d_model contraction
QKV = (NH + 2) * D  # 768 qkv rows per core
NT = 4            # 512-wide q/T tiles
TQ = 512
EPS = 1e-6
SM_SCALE = 1.0 / float(np.sqrt(D))
EXP_BIAS = -2.0   # exp(s*scale - 2): keeps P within e4m3 range
WSCALE = 16.0     # host-side W_qkv scale (e4m3 normal-range), undone via
                  # RMS-norm scale-invariance (q,k) and W_proj/16 (v)


def build_kernel():
    nc = bacc.Bacc()
    xT_d = nc.dram_tensor("xT", [C, T], F8, kind="ExternalInput")
    xT16_d = nc.dram_tensor("xT16", [C, TQ // 2], MM_DT, kind="ExternalInput")
    wg_d = nc.dram_tensor("wg", [C, QKV], F8, kind="ExternalInput")
    wg16_d = nc.dram_tensor("wg16", [C, QKV], MM_DT, kind="ExternalInput")
    wp_d = nc.dram_tensor("wp", [NH * D, C], MM_DT, kind="ExternalInput")
    # rms-norm folded constants (host-computed, see make_in_maps):
    # sqrt(ssq*qsc + qbi) = 16/qs * sqrt(mean+eps), so its reciprocal
    # times the 16x-scaled accumulator directly yields qs * rms_norm(q)
    qsc_d = nc.dram_tensor("qsc", [D, 1], F32, kind="ExternalInput")
    qbi_d = nc.dram_tensor("qbi", [D, 1], F32, kind="ExternalInput")
    ksc_d = nc.dram_tensor("ksc", [D, 1], F32, kind="ExternalInput")
    kbi_d = nc.dram_tensor("kbi", [D, 1], F32, kind="ExternalInput")
    out_d = nc.dram_tensor("out", [T, C], F32, kind="ExternalOutput")

    xT_r = xT_d.rearrange("(kc p) t -> p kc t", p=128)
    xT16_r = xT16_d.rearrange("(kc p) t -> p kc t", p=128)
    wg_r = wg_d.rearrange("(kc p) q -> p kc q", p=128)
    wg16_r = wg16_d.rearrange("(kc p) q -> p kc q", p=128)

    with tile.TileContext(nc) as tc:
        with (
            tc.tile_pool(name="consts", bufs=1) as consts,
            tc.tile_pool(name="qkv_sb", bufs=1) as qkv_sb,
        ):
            # ---- constants ----
            ident = consts.tile([128, 128], MM_DT)
            make_identity(nc, ident)
            ones32 = consts.tile([128, 128], F32)
            nc.vector.memset(ones32, 1.0)
            ones_m = consts.tile([128, 128], MM_DT)
            nc.vector.tensor_copy(ones_m, ones32)
            ones8 = consts.tile([128, 2, 128], F8)
            nc.vector.tensor_copy(ones8[:, 0, :], ones32)
            nc.vector.tensor_copy(ones8[:, 1, :], ones32)
            eps_t = consts.tile([128, 1], F32)
            nc.vector.memset(eps_t, EPS)
            nb_t = consts.tile([128, 1], F32)
            nc.vector.memset(nb_t, EXP_BIAS)
            qsc_t = consts.tile([128, 1], F32)
            qbi_t = consts.tile([128, 1], F32)
            ksc_t = consts.tile([128, 1], F32)
            kbi_t = consts.tile([128, 1], F32)
            nc.sync.dma_start(out=qsc_t, in_=qsc_d[:, :])
            nc.sync.dma_start(out=qbi_t, in_=qbi_d[:, :])
            nc.sync.dma_start(out=ksc_t, in_=ksc_d[:, :])
            nc.sync.dma_start(out=kbi_t, in_=kbi_d[:, :])

            # ---- persistent activations, split per 512-wide tile so the
            # attention on tile n only depends on stage A tile n ----
            qTn = [
                qkv_sb.tile([128, NH, TQ], MM_DT, name=f"qT{n}")
                for n in range(NT)
            ]
            kTn = [
                qkv_sb.tile([128, TQ], MM_DT, name=f"kT{n}")
                for n in range(NT)
            ]
            vn = [
                qkv_sb.tile([128, 4, 128], MM_DT, name=f"v{n}")
                for n in range(NT)
            ]
            v8n = [
                qkv_sb.tile([128, 4, 128], F8, name=f"v8{n}")
                for n in range(NT)
            ]
            oTn = [
                qkv_sb.tile([128, NH, TQ], MM_DT, name=f"oT{n}")
                for n in range(NT)
            ]
            wp_sb = qkv_sb.tile([128, NH, C], MM_DT)
            wp_r = wp_d.rearrange("(h p) c -> p h c", p=128)

            # ================= Stage A: qkvT = Wg.T @ xT =================
            with (
                tc.tile_pool(name="wg_pool", bufs=1) as wg_pool,
                tc.tile_pool(name="xt_pool", bufs=2) as xt_pool,
                tc.tile_pool(name="normtmp", bufs=4) as normtmp,
                tc.tile_pool(name="vtmp", bufs=2) as vtmp,
                tc.tile_pool(name="psA", bufs=1, space="PSUM") as psA,
                tc.tile_pool(name="psN", bufs=1, space="PSUM") as psN,
                tc.tile_pool(name="psV", bufs=1, space="PSUM") as psV,
            ):
                # per-chunk loads so the first matmuls start early.
                # tile 0 (positions 0-511) runs bf16: its q/k/v feed the
                # low-key-count rows where softmax averaging cannot wash
                # out fp8 input noise.  tiles 1-3 run fp8 DoubleRow.
                wg_sb = wg_pool.tile([128, NKC, QKV], F8)
                wg16_sb = wg_pool.tile([128, NKC, QKV], MM_DT)
                # tile 0 splits: cols 0:256 bf16 (lowest-key-count rows),
                # cols 256:512 fp8 DR
                xt0_sb = wg_pool.tile([128, NKC, TQ // 2], MM_DT)
                xt0r_sb = wg_pool.tile([128, NKC, TQ // 2], F8)
                xts = [xt0_sb]
                for n in range(1, NT):
                    xts.append(
                        xt_pool.tile(
                            [128, NKC, TQ], F8, tag="xt", name=f"xt{n}"
                        )
                    )
                # first chunks land individually so compute starts early;
                # the rest ride in 4-chunk batches (sync-engine descriptor
                # issue costs ~0.6us each).  fp8 weights interleave so tile
                # 1 is fed by the time tile 0's bf16 compute finishes.
                for kc in range(4):
                    nc.sync.dma_start(
                        out=xt0_sb[:, kc, :], in_=xT16_r[:, kc, :]
                    )
                    nc.sync.dma_start(
                        out=xt0r_sb[:, kc, :], in_=xT_r[:, kc, 256:TQ]
                    )
                    nc.sync.dma_start(
                        out=wg16_sb[:, kc, :], in_=wg16_r[:, kc, :]
                    )
                    nc.sync.dma_start(
                        out=wg_sb[:, kc, :], in_=wg_r[:, kc, :]
                    )
                for g4 in range(1, 4):
                    k0 = 4 * g4
                    nc.sync.dma_start(
                        out=xt0_sb[:, k0:k0 + 4, :],
                        in_=xT16_r[:, k0:k0 + 4, :],
                    )
                    nc.sync.dma_start(
                        out=xt0r_sb[:, k0:k0 + 4, :],
                        in_=xT_r[:, k0:k0 + 4, 256:TQ],
                    )
                    nc.sync.dma_start(
                        out=wg16_sb[:, k0:k0 + 4, :],
                        in_=wg16_r[:, k0:k0 + 4, :],
                    )
                    nc.sync.dma_start(
                        out=wg_sb[:, k0:k0 + 4, :],
                        in_=wg_r[:, k0:k0 + 4, :],
                    )
                for n in range(NT):
                    xt_sb = xts[n]
                    if n + 1 < NT:
                        for g4 in range(4):
                            nc.sync.dma_start(
                                out=xts[n + 1][:, 4 * g4:4 * g4 + 4, :],
                                in_=xT_r[
                                    :, 4 * g4:4 * g4 + 4,
                                    (n + 1) * TQ:(n + 2) * TQ
                                ],
                            )
                    if n == 1:
                        # wp prefetch: late enough not to delay xt tiles,
                        # early enough to be resident before proj starts
                        for h in range(NH):
                            nc.sync.dma_start(
                                out=wp_sb[:, h, :], in_=wp_r[:, h, :]
                            )
                    accs = [
                        psA.tile([128, TQ], F32, tag=f"acc{m}", name=f"acc{m}")
                        for m in range(6)
                    ]
                    if n == 0:
                        for kc in range(NKC):
                            for m in range(6):
                                nc.tensor.matmul(
                                    accs[m][:, 0:TQ // 2],
                                    lhsT=wg16_sb[
                                        :, kc, m * 128:(m + 1) * 128
                                    ],
                                    rhs=xt0_sb[:, kc, :],
                                    start=(kc == 0),
                                    stop=(kc == NKC - 1),
                                )
                        for kc2 in range(NKC // 2):
                            for m in range(6):
                                nc.tensor.matmul(
                                    accs[m][:, TQ // 2:TQ],
                                    lhsT=wg_sb[
                                        :, 2 * kc2:2 * kc2 + 2,
                                        m * 128:(m + 1) * 128
                                    ],
                                    rhs=xt0r_sb[:, 2 * kc2:2 * kc2 + 2, :],
                                    start=(kc2 == 0),
                                    stop=(kc2 == NKC // 2 - 1),
                                    perf_mode=DR,
                                )
                    else:
                        # kc-pair DoubleRow: chunks into 6 accumulators
                        for kc2 in range(NKC // 2):
                            for m in range(6):
                                nc.tensor.matmul(
                                    accs[m],
                                    lhsT=wg_sb[
                                        :, 2 * kc2:2 * kc2 + 2,
                                        m * 128:(m + 1) * 128
                                    ],
                                    rhs=xt_sb[:, 2 * kc2:2 * kc2 + 2, :],
                                    start=(kc2 == 0),
                                    stop=(kc2 == NKC // 2 - 1),
                                    perf_mode=DR,
                                )
                    for m in range(6):
                        acc = accs[m]
                        if m < 5:
                            # rms over partition dim via ones-matmul bcast;
                            # acc is 16x raw, so square at 1/16 input scale
                            sq = normtmp.tile([128, TQ], MM_DT, tag="sq")
                            nc.scalar.activation(
                                out=sq, in_=acc,
                                func=mybir.ActivationFunctionType.Square,
                                scale=1.0 / WSCALE,
                            )
                            ssq = psN.tile([128, TQ], F32, tag="ssq")
                            nc.tensor.matmul(ssq, lhsT=ones_m, rhs=sq)
                            # per-partition scale/bias fold qs and the 16x
                            # weight scale into one sqrt+recip+mul chain
                            rms = normtmp.tile([128, TQ], F32, tag="rms")
                            nc.scalar.activation(
                                out=rms, in_=ssq,
                                func=mybir.ActivationFunctionType.Sqrt,
                                bias=qbi_t if m < 4 else kbi_t,
                                scale=qsc_t if m < 4 else ksc_t,
                            )
                            rinv = normtmp.tile([128, TQ], F32, tag="rinv")
                            nc.vector.reciprocal_approx_fast(out=rinv, in_=rms)
                            dst = qTn[n][:, m, :] if m < 4 else kTn[n][:, :]
                            nc.vector.tensor_mul(dst, acc, rinv)
                        else:
                            # v block: transpose to natural [tk, d]
                            vt = vtmp.tile([128, TQ], MM_DT, tag="vt")
                            nc.vector.tensor_copy(vt, acc)
                            for jj in range(4):
                                vps = psV.tile([128, 128], MM_DT, tag="vps")
                                nc.tensor.transpose(
                                    vps, vt[:, jj * 128:(jj + 1) * 128], ident
                                )
                                nc.vector.tensor_copy(vn[n][:, jj, :], vps)
                                nc.vector.tensor_copy(v8n[n][:, jj, :], vps)
                    if n == NT - 1:
                        # preload the exp table set now, overlapping the
                        # final norm chain instead of the first S batch
                        warm = normtmp.tile([128, 1], F32, tag="warm")
                        nc.scalar.activation(
                            out=warm, in_=eps_t,
                            func=mybir.ActivationFunctionType.Exp,
                        )

            # ============ Attention + proj, per 512-wide q tile ============
            with (
                tc.tile_pool(name="pt_pool", bufs=4) as pt_pool,
                tc.tile_pool(name="rs_pool", bufs=3) as rs_pool,
                tc.tile_pool(name="yp_pool", bufs=2) as yp_pool,
                tc.tile_pool(name="psS", bufs=2, space="PSUM") as psS,
                tc.tile_pool(name="psO", bufs=1, space="PSUM") as psO,
                tc.tile_pool(name="psU", bufs=1, space="PSUM") as psU,
                tc.tile_pool(name="psC", bufs=2, space="PSUM") as psC,
            ):
                def emit_proj(tqt):
                    # y[tq tile] = (oT).T @ wp for this 512-wide q tile;
                    # copy + DMA per 512-col chunk so the output drain
                    # pipelines with the next chunk's matmuls
                    for tt in range(4):
                        c0 = tqt * TQ + tt * 128
                        yp = yp_pool.tile([128, C], F32, tag="yp", name="yp")
                        for cn in range(4):
                            pc = psC.tile([128, TQ], F32, tag="pc", name="pc")
                            for h in range(NH):
                                nc.tensor.matmul(
                                    pc,
                                    lhsT=oTn[tqt][:, h, c0 - tqt * TQ:c0 - tqt * TQ + 128],
                                    rhs=wp_sb[:, h, cn * TQ:(cn + 1) * TQ],
                                    start=(h == 0),
                                    stop=(h == NH - 1),
                                )
                            if cn == 0:
                                nc.scalar.copy(yp[:, cn * TQ:(cn + 1) * TQ], pc)
                            else:
                                nc.vector.tensor_copy(
                                    yp[:, cn * TQ:(cn + 1) * TQ], pc
                                )
                            nc.sync.dma_start(
                                out=out_d[c0:c0 + 128, cn * TQ:(cn + 1) * TQ],
                                in_=yp[:, cn * TQ:(cn + 1) * TQ],
                            )

                def flush(item):
                    # PV + row-sum matmuls for a finished exp unit;
                    # normalize the head after its last unit; emit proj
                    # for the q tile after its last head.
                    (kind, payload, o_ps, u_ps, tqt, h,
                     is_first, is_last) = item
                    if kind == "pair":
                        i, p8 = payload
                        j0 = 2 * i
                        nc.tensor.matmul(
                            o_ps,
                            lhsT=v8n[j0 // 4][:, j0 % 4:j0 % 4 + 2, :],
                            rhs=p8,
                            start=is_first,
                            stop=False,
                            perf_mode=DR,
                        )
                        nc.tensor.matmul(
                            u_ps,
                            lhsT=ones8,
                            rhs=p8,
                            start=is_first,
                            stop=False,
                            perf_mode=DR,
                        )
                    else:
                        batch, p_sb, first_in_batch = payload
                        for bi, (jj, co, w, pos) in enumerate(batch):
                            st = is_first and first_in_batch and bi == 0
                            sp = is_last and bi == len(batch) - 1
                            nc.tensor.matmul(
                                o_ps[:, co:TQ],
                                lhsT=vn[tqt][:, jj, :],
                                rhs=p_sb[:, pos:pos + w],
                                start=st,
                                stop=sp,
                            )
                            nc.tensor.matmul(
                                u_ps[:, co:TQ],
                                lhsT=ones_m,
                                rhs=p_sb[:, pos:pos + w],
                                start=st,
                                stop=sp,
                            )
                    if is_last:
                        rsum = rs_pool.tile(
                            [128, TQ], F32, tag="rsum", name="rsum"
                        )
                        nc.vector.reciprocal_approx_fast(out=rsum, in_=u_ps)
                        nc.vector.tensor_mul(
                            oTn[tqt][:, h, :], o_ps, rsum
                        )
                        if h == NH - 1:
                            emit_proj(tqt)

                # diagonal-chunk packing: widths 512,384 | 256,128
                DIAG_BATCHES = [
                    [(0, 0, 512, 0), (1, 128, 384, 512)],
                    [(2, 256, 256, 0), (3, 384, 128, 256)],
                ]

                pending = None
                for tqt in range(NT):
                    for h in range(NH):
                        o_ps = psO.tile([128, TQ], F32, tag="o", name="o_ps")
                        u_ps = psU.tile([128, TQ], F32, tag="u", name="u_ps")
                        n_units = 2 * tqt + 2
                        ui = 0
                        # full-width off-diagonal key-chunk pairs (fp8 DR)
                        for i in range(2 * tqt):
                            s3 = psS.tile(
                                [128, 2, TQ], F32, tag="s", name="s_ps"
                            )
                            for ii in range(2):
                                j = 2 * i + ii
                                nc.tensor.matmul(
                                    s3[:, ii, :],
                                    lhsT=kTn[j // 4][
                                        :, (j % 4) * 128:(j % 4 + 1) * 128
                                    ],
                                    rhs=qTn[tqt][:, h, :],
                                )
                            p8 = pt_pool.tile(
                                [128, 2, TQ], F8, tag="p8", name="p8"
                            )
                            nc.scalar.activation(
                                out=p8, in_=s3,
                                func=mybir.ActivationFunctionType.Exp,
                                scale=SM_SCALE, bias=nb_t,
                            )
                            if pending is not None:
                                flush(pending)
                            pending = (
                                "pair", (i, p8), o_ps, u_ps, tqt, h,
                                ui == 0, False,
                            )
                            ui += 1
                        # diagonal chunks (bf16, causal-masked)
                        for bnum, batch in enumerate(DIAG_BATCHES):
                            bw = batch[-1][3] + batch[-1][2]
                            s_ps = psS.tile(
                                [128, 2, TQ], F32, tag="s", name="s_ps"
                            )
                            s2 = s_ps.rearrange("p a b -> p (a b)")
                            for (jj, co, w, pos) in batch:
                                nc.tensor.matmul(
                                    s2[:, pos:pos + w],
                                    lhsT=kTn[tqt][
                                        :, jj * 128:(jj + 1) * 128
                                    ],
                                    rhs=qTn[tqt][:, h, co:TQ],
                                )
                            p_sb = pt_pool.tile(
                                [128, 1024], MM_DT, tag="p", name="p_sb"
                            )
                            nc.scalar.activation(
                                out=p_sb[:, 0:bw], in_=s2[:, 0:bw],
                                func=mybir.ActivationFunctionType.Exp,
                                scale=SM_SCALE, bias=nb_t,
                            )
                            for (jj, co, w, pos) in batch:
                                # causal mask: zero p where col < row
                                # (gpsimd: otherwise-idle engine)
                                nc.gpsimd.affine_select(
                                    out=p_sb[:, pos:pos + 128],
                                    in_=p_sb[:, pos:pos + 128],
                                    pattern=[[1, 128]],
                                    channel_multiplier=-1, base=0,
                                    compare_op=mybir.AluOpType.is_ge,
                                    fill=0.0,
                                )
                            if pending is not None:
                                flush(pending)
                            pending = (
                                "diag", (batch, p_sb, bnum == 0),
                                o_ps, u_ps, tqt, h,
                                ui == 0, bnum == 1,
                            )
                            ui += 1
                if pending is not None:
                    flush(pending)

    nc.finalize()
    return nc


@functools.lru_cache(maxsize=1)
def _get_nc():
    return build_kernel()


def make_in_maps(x, W_qkv, W_proj, q_scale, k_scale):
    x = np.asarray(x, dtype=np.float32)
    W_qkv = np.asarray(W_qkv, dtype=np.float32)
    W_proj = np.asarray(W_proj, dtype=np.float32)
    q_scale = np.asarray(q_scale, dtype=np.float32)
    k_scale = np.asarray(k_scale, dtype=np.float32)

    import ml_dtypes

    bf16 = ml_dtypes.bfloat16
    f8 = ml_dtypes.float8_e4m3
    # rms-norm folded constants: sqrt(ssq*qsc + qbi) = (16/qs)*sqrt(mean+eps)
    ws2 = WSCALE * WSCALE
    with np.errstate(divide="ignore"):
        qsc = np.ascontiguousarray(
            (ws2 / (D * q_scale**2)).reshape(D, 1)).astype(np.float32)
        qbi = np.ascontiguousarray(
            (ws2 * EPS / q_scale**2).reshape(D, 1)).astype(np.float32)
        ksc = np.ascontiguousarray(
            (ws2 / (D * k_scale**2)).reshape(D, 1)).astype(np.float32)
        kbi = np.ascontiguousarray(
            (ws2 * EPS / k_scale**2).reshape(D, 1)).astype(np.float32)
    xT_by_batch = [
        np.clip(np.ascontiguousarray(x[b].T), -224, 224).astype(f8)
        for b in range(2)
    ]
    xT16_by_batch = [
        np.ascontiguousarray(x[b].T[:, 0:TQ // 2]).astype(bf16)
        for b in range(2)
    ]
    in_maps = []
    for core in range(8):
        b, g = divmod(core, 4)
        wg_full = np.ascontiguousarray(
            np.concatenate(
                [
                    W_qkv[:, 512 * g:512 * (g + 1)],
                    W_qkv[:, 2048 + 128 * g:2048 + 128 * (g + 1)],
                    W_qkv[:, 2560 + 128 * g:2560 + 128 * (g + 1)],
                ],
                axis=1,
            )
        ) * WSCALE
        wg = np.clip(wg_full, -224, 224).astype(f8)
        wg16 = wg_full.astype(bf16)
        wp = np.ascontiguousarray(
            W_proj[512 * g:512 * (g + 1), :] / WSCALE
        ).astype(bf16)
        in_maps.append(
            {
                "xT": xT_by_batch[b], "xT16": xT16_by_batch[b],
                "wg": wg, "wg16": wg16, "wp": wp,
                "qsc": qsc, "qbi": qbi, "ksc": ksc, "kbi": kbi,
            }
        )
    return in_maps


def kernel(x, W_qkv, W_proj, q_scale, k_scale):
    nc = _get_nc()
    in_maps = make_in_maps(x, W_qkv, W_proj, q_scale, k_scale)
    res = run_bass_kernel_spmd(nc, in_maps, core_ids=list(range(8)))
    outs = [r["out"] for r in res.results]
    y0 = outs[0] + outs[1] + outs[2] + outs[3]
    y1 = outs[4] + outs[5] + outs[6] + outs[7]
    return np.stack([y0, y1], axis=0).astype(np.float32)


# revision 25
# speedup vs baseline: 1.0923x; 1.0923x over previous
"""Trainium2 Bass kernel for causal GQA self-attention with QK RMS-norm.

Problem (hardcoded): B=2, T=2048, d_model=2048, 16 Q heads / 4 KV heads,
head_dim=128, fp32 IO.

Sharding across 8 NeuronCores: tensor-parallel over the 4 KV head groups
(each group = 1 KV head + its 4 Q heads) x data-parallel over the 2
batches.  core = 4*b + g.  Each core computes
    qkvT_g = Wg.T @ x_b.T          ([768, T])
    q/k RMS-norm (+ per-dim scales), v transposed to natural layout
    causal attention for the 4 Q heads of group g (S^T orientation)
    yp_partial = (O^T).T @ Wp_g    ([T, d_model] partial)
and the host sums the 4 partials per batch.

fp8 scheme: W_qkv is pre-scaled by 16 on the host so its entries sit in
e4m3's normal range; q/k RMS-norm is scale-invariant (per-dim scales are
divided by 16 to compensate), and v's 16x is folded into W_proj/16 (bf16).
The QKV projection and the full-width (off-diagonal) PV + row-sum matmuls
run as fp8 DoubleRow (contraction pairs, 2x PE throughput).  exp carries a
-2 bias so P fits e4m3 (ratio O/rowsum is bias-invariant).  Diagonal
blocks and the output projection keep the bf16 path.
"""

import functools

import numpy as np

import concourse.bass as bass
import concourse.mybir as mybir
import concourse.tile as tile
from concourse import bacc
from concourse.bass_utils import run_bass_kernel_spmd
from concourse.masks import make_identity

F32 = mybir.dt.float32
BF16 = mybir.dt.bfloat16
F8 = mybir.dt.float8e4
DR = mybir.MatmulPerfMode.DoubleRow

MM_DT = BF16

T = 2048
C = 2048
D = 128
NH = 4            # q heads per core
NKC = C // 128    # 16 k-chunks of the d_model contraction
QKV = (NH + 2) * D  # 768 qkv rows per core
NT = 4            # 512-wide q/T tiles
TQ = 512
EPS = 1e-6
SM_SCALE = 1.0 / float(np.sqrt(D))
EXP_BIAS = -2.0   # exp(s*scale - 2): keeps P within e4m3 range
WSCALE = 16.0     # host-side W_qkv scale (e4m3 normal-range), undone via
                  # RMS-norm scale-invariance (q,k) and W_proj/16 (v)


def build_kernel():
    nc = bacc.Bacc()
    xT_d = nc.dram_tensor("xT", [C, T], F8, kind="ExternalInput")
    xT16_d = nc.dram_tensor("xT16", [C, TQ], MM_DT, kind="ExternalInput")
    wg_d = nc.dram_tensor("wg", [C, QKV], F8, kind="ExternalInput")
    wg16_d = nc.dram_tensor("wg16", [C, QKV], MM_DT, kind="ExternalInput")
    wp_d = nc.dram_tensor("wp", [NH * D, C], MM_DT, kind="ExternalInput")
    # rms-norm folded constants (host-computed, see make_in_maps):
    # sqrt(ssq*qsc + qbi) = 16/qs * sqrt(mean+eps), so its reciprocal
    # times the 16x-scaled accumulator directly yields qs * rms_norm(q)
    qsc_d = nc.dram_tensor("qsc", [D, 1], F32, kind="ExternalInput")
    qbi_d = nc.dram_tensor("qbi", [D, 1], F32, kind="ExternalInput")
    ksc_d = nc.dram_tensor("ksc", [D, 1], F32, kind="ExternalInput")
    kbi_d = nc.dram_tensor("kbi", [D, 1], F32, kind="ExternalInput")
    out_d = nc.dram_tensor("out", [T, C], F32, kind="ExternalOutput")

    xT_r = xT_d.rearrange("(kc p) t -> p kc t", p=128)
    xT16_r = xT16_d.rearrange("(kc p) t -> p kc t", p=128)
    wg_r = wg_d.rearrange("(kc p) q -> p kc q", p=128)
    wg16_r = wg16_d.rearrange("(kc p) q -> p kc q", p=128)

    with tile.TileContext(nc) as tc:
        with (
            tc.tile_pool(name="consts", bufs=1) as consts,
            tc.tile_pool(name="qkv_sb", bufs=1) as qkv_sb,
        ):
            # ---- constants ----
            ident = consts.tile([128, 128], MM_DT)
            make_identity(nc, ident)
            ones32 = consts.tile([128, 128], F32)
            nc.vector.memset(ones32, 1.0)
            ones_m = consts.tile([128, 128], MM_DT)
            nc.vector.tensor_copy(ones_m, ones32)
            ones8 = consts.tile([128, 2, 128], F8)
            nc.vector.tensor_copy(ones8[:, 0, :], ones32)
            nc.vector.tensor_copy(ones8[:, 1, :], ones32)
            eps_t = consts.tile([128, 1], F32)
            nc.vector.memset(eps_t, EPS)
            nb_t = consts.tile([128, 1], F32)
            nc.vector.memset(nb_t, EXP_BIAS)
            qsc_t = consts.tile([128, 1], F32)
            qbi_t = consts.tile([128, 1], F32)
            ksc_t = consts.tile([128, 1], F32)
            kbi_t = consts.tile([128, 1], F32)
            nc.sync.dma_start(out=qsc_t, in_=qsc_d[:, :])
            nc.sync.dma_start(out=qbi_t, in_=qbi_d[:, :])
            nc.sync.dma_start(out=ksc_t, in_=ksc_d[:, :])
            nc.sync.dma_start(out=kbi_t, in_=kbi_d[:, :])

            # ---- persistent activations, split per 512-wide tile so the
            # attention on tile n only depends on stage A tile n ----
            qTn = [
                qkv_sb.tile([128, NH, TQ], MM_DT, name=f"qT{n}")
                for n in range(NT)
            ]
            kTn = [
                qkv_sb.tile([128, TQ], MM_DT, name=f"kT{n}")
                for n in range(NT)
            ]
            vn = [
                qkv_sb.tile([128, 4, 128], MM_DT, name=f"v{n}")
                for n in range(NT)
            ]
            v8n = [
                qkv_sb.tile([128, 4, 128], F8, name=f"v8{n}")
                for n in range(NT)
            ]
            oTn = [
                qkv_sb.tile([128, NH, TQ], MM_DT, name=f"oT{n}")
                for n in range(NT)
            ]
            wp_sb = qkv_sb.tile([128, NH, C], MM_DT)
            wp_r = wp_d.rearrange("(h p) c -> p h c", p=128)

            # ================= Stage A: qkvT = Wg.T @ xT =================
            with (
                tc.tile_pool(name="wg_pool", bufs=1) as wg_pool,
                tc.tile_pool(name="xt_pool", bufs=2) as xt_pool,
                tc.tile_pool(name="normtmp", bufs=4) as normtmp,
                tc.tile_pool(name="vtmp", bufs=2) as vtmp,
                tc.tile_pool(name="psA", bufs=1, space="PSUM") as psA,
                tc.tile_pool(name="psN", bufs=1, space="PSUM") as psN,
                tc.tile_pool(name="psV", bufs=1, space="PSUM") as psV,
            ):
                # per-chunk loads so the first matmuls start early.
                # tile 0 (positions 0-511) runs bf16: its q/k/v feed the
                # low-key-count rows where softmax averaging cannot wash
                # out fp8 input noise.  tiles 1-3 run fp8 DoubleRow.
                wg_sb = wg_pool.tile([128, NKC, QKV], F8)
                wg16_sb = wg_pool.tile([128, NKC, QKV], MM_DT)
                xt0_sb = wg_pool.tile([128, NKC, TQ], MM_DT)
                xts = [xt0_sb]
                for n in range(1, NT):
                    xts.append(
                        xt_pool.tile(
                            [128, NKC, TQ], F8, tag="xt", name=f"xt{n}"
                        )
                    )
                # first chunks land individually so compute starts early;
                # the rest ride in 4-chunk batches (sync-engine descriptor
                # issue costs ~0.6us each).  fp8 weights interleave so tile
                # 1 is fed by the time tile 0's bf16 compute finishes.
                for kc in range(4):
                    nc.sync.dma_start(
                        out=xt0_sb[:, kc, :], in_=xT16_r[:, kc, :]
                    )
                    nc.sync.dma_start(
                        out=wg16_sb[:, kc, :], in_=wg16_r[:, kc, :]
                    )
                    nc.sync.dma_start(
                        out=wg_sb[:, kc, :], in_=wg_r[:, kc, :]
                    )
                for g4 in range(1, 4):
                    k0 = 4 * g4
                    nc.sync.dma_start(
                        out=xt0_sb[:, k0:k0 + 4, :],
                        in_=xT16_r[:, k0:k0 + 4, :],
                    )
                    nc.sync.dma_start(
                        out=wg16_sb[:, k0:k0 + 4, :],
                        in_=wg16_r[:, k0:k0 + 4, :],
                    )
                    nc.sync.dma_start(
                        out=wg_sb[:, k0:k0 + 4, :],
                        in_=wg_r[:, k0:k0 + 4, :],
                    )
                for n in range(NT):
                    xt_sb = xts[n]
                    if n + 1 < NT:
                        for g4 in range(4):
                            nc.sync.dma_start(
                                out=xts[n + 1][:, 4 * g4:4 * g4 + 4, :],
                                in_=xT_r[
                                    :, 4 * g4:4 * g4 + 4,
                                    (n + 1) * TQ:(n + 2) * TQ
                                ],
                            )
                    if n == 1:
                        # wp prefetch: late enough not to delay xt tiles,
                        # early enough to be resident before proj starts
                        for h in range(NH):
                            nc.sync.dma_start(
                                out=wp_sb[:, h, :], in_=wp_r[:, h, :]
                            )
                    accs = [
                        psA.tile([128, TQ], F32, tag=f"acc{m}", name=f"acc{m}")
                        for m in range(6)
                    ]
                    if n == 0:
                        for kc in range(NKC):
                            for m in range(6):
                                nc.tensor.matmul(
                                    accs[m],
                                    lhsT=wg16_sb[
                                        :, kc, m * 128:(m + 1) * 128
                                    ],
                                    rhs=xt0_sb[:, kc, :],
                                    start=(kc == 0),
                                    stop=(kc == NKC - 1),
                                )
                    else:
                        # kc-pair DoubleRow: chunks into 6 accumulators
                        for kc2 in range(NKC // 2):
                            for m in range(6):
                                nc.tensor.matmul(
                                    accs[m],
                                    lhsT=wg_sb[
                                        :, 2 * kc2:2 * kc2 + 2,
                                        m * 128:(m + 1) * 128
                                    ],
                                    rhs=xt_sb[:, 2 * kc2:2 * kc2 + 2, :],
                                    start=(kc2 == 0),
                                    stop=(kc2 == NKC // 2 - 1),
                                    perf_mode=DR,
                                )
                    for m in range(6):
                        acc = accs[m]
                        if m < 5:
                            # rms over partition dim via ones-matmul bcast;
                            # acc is 16x raw, so square at 1/16 input scale
                            sq = normtmp.tile([128, TQ], MM_DT, tag="sq")
                            nc.scalar.activation(
                                out=sq, in_=acc,
                                func=mybir.ActivationFunctionType.Square,
                                scale=1.0 / WSCALE,
                            )
                            ssq = psN.tile([128, TQ], F32, tag="ssq")
                            nc.tensor.matmul(ssq, lhsT=ones_m, rhs=sq)
                            # per-partition scale/bias fold qs and the 16x
                            # weight scale into one sqrt+recip+mul chain
                            rms = normtmp.tile([128, TQ], F32, tag="rms")
                            nc.scalar.activation(
                                out=rms, in_=ssq,
                                func=mybir.ActivationFunctionType.Sqrt,
                                bias=qbi_t if m < 4 else kbi_t,
                                scale=qsc_t if m < 4 else ksc_t,
                            )
                            rinv = normtmp.tile([128, TQ], F32, tag="rinv")
                            nc.vector.reciprocal_approx_fast(out=rinv, in_=rms)
                            dst = qTn[n][:, m, :] if m < 4 else kTn[n][:, :]
                            nc.vector.tensor_mul(dst, acc, rinv)
                        else:
                            # v block: transpose to natural [tk, d]
                            vt = vtmp.tile([128, TQ], MM_DT, tag="vt")
                            nc.vector.tensor_copy(vt, acc)
                            for jj in range(4):
                                vps = psV.tile([128, 128], MM_DT, tag="vps")
                                nc.tensor.transpose(
                                    vps, vt[:, jj * 128:(jj + 1) * 128], ident
                                )
                                nc.vector.tensor_copy(vn[n][:, jj, :], vps)
                                nc.vector.tensor_copy(v8n[n][:, jj, :], vps)
                    if n == NT - 1:
                        # preload the exp table set now, overlapping the
                        # final norm chain instead of the first S batch
                        warm = normtmp.tile([128, 1], F32, tag="warm")
                        nc.scalar.activation(
                            out=warm, in_=eps_t,
                            func=mybir.ActivationFunctionType.Exp,
                        )

            # ============ Attention + proj, per 512-wide q tile ============
            with (
                tc.tile_pool(name="pt_pool", bufs=4) as pt_pool,
                tc.tile_pool(name="rs_pool", bufs=3) as rs_pool,
                tc.tile_pool(name="yp_pool", bufs=2) as yp_pool,
                tc.tile_pool(name="psS", bufs=2, space="PSUM") as psS,
                tc.tile_pool(name="psO", bufs=1, space="PSUM") as psO,
                tc.tile_pool(name="psU", bufs=1, space="PSUM") as psU,
                tc.tile_pool(name="psC", bufs=2, space="PSUM") as psC,
            ):
                def emit_proj(tqt):
                    # y[tq tile] = (oT).T @ wp for this 512-wide q tile;
                    # copy + DMA per 512-col chunk so the output drain
                    # pipelines with the next chunk's matmuls
                    for tt in range(4):
                        c0 = tqt * TQ + tt * 128
                        yp = yp_pool.tile([128, C], F32, tag="yp", name="yp")
                        for cn in range(4):
                            pc = psC.tile([128, TQ], F32, tag="pc", name="pc")
                            for h in range(NH):
                                nc.tensor.matmul(
                                    pc,
                                    lhsT=oTn[tqt][:, h, c0 - tqt * TQ:c0 - tqt * TQ + 128],
                                    rhs=wp_sb[:, h, cn * TQ:(cn + 1) * TQ],
                                    start=(h == 0),
                                    stop=(h == NH - 1),
                                )
                            if cn == 0:
                                nc.scalar.copy(yp[:, cn * TQ:(cn + 1) * TQ], pc)
                            else:
                                nc.vector.tensor_copy(
                                    yp[:, cn * TQ:(cn + 1) * TQ], pc
                                )
                            nc.sync.dma_start(
                                out=out_d[c0:c0 + 128, cn * TQ:(cn + 1) * TQ],
                                in_=yp[:, cn * TQ:(cn + 1) * TQ],
                            )

                def flush(item):
                    # PV + row-sum matmuls for a finished exp unit;
                    # normalize the head after its last unit; emit proj
                    # for the q tile after its last head.
                    (kind, payload, o_ps, u_ps, tqt, h,
                     is_first, is_last) = item
                    if kind == "pair":
                        i, p8 = payload
                        j0 = 2 * i
                        nc.tensor.matmul(
                            o_ps,
                            lhsT=v8n[j0 // 4][:, j0 % 4:j0 % 4 + 2, :],
                            rhs=p8,
                            start=is_first,
                            stop=False,
                            perf_mode=DR,
                        )
                        nc.tensor.matmul(
                            u_ps,
                            lhsT=ones8,
                            rhs=p8,
                            start=is_first,
                            stop=False,
                            perf_mode=DR,
                        )
                    else:
                        batch, p_sb, first_in_batch = payload
                        for bi, (jj, co, w, pos) in enumerate(batch):
                            st = is_first and first_in_batch and bi == 0
                            sp = is_last and bi == len(batch) - 1
                            nc.tensor.matmul(
                                o_ps[:, co:TQ],
                                lhsT=vn[tqt][:, jj, :],
                                rhs=p_sb[:, pos:pos + w],
                                start=st,
                                stop=sp,
                            )
                            nc.tensor.matmul(
                                u_ps[:, co:TQ],
                                lhsT=ones_m,
                                rhs=p_sb[:, pos:pos + w],
                                start=st,
                                stop=sp,
                            )
                    if is_last:
                        rsum = rs_pool.tile(
                            [128, TQ], F32, tag="rsum", name="rsum"
                        )
                        nc.vector.reciprocal_approx_fast(out=rsum, in_=u_ps)
                        nc.vector.tensor_mul(
                            oTn[tqt][:, h, :], o_ps, rsum
                        )
                        if h == NH - 1:
                            emit_proj(tqt)

                # diagonal-chunk packing: widths 512,384 | 256,128
                DIAG_BATCHES = [
                    [(0, 0, 512, 0), (1, 128, 384, 512)],
                    [(2, 256, 256, 0), (3, 384, 128, 256)],
                ]

                pend = []

                def push(item):
                    # 2-deep software pipeline: PV/row-sum of unit k issue
                    # after unit k+2's exp, decoupling tensor from ACT
                    if len(pend) == 2:
                        flush(pend.pop(0))
                    pend.append(item)
                for tqt in range(NT):
                    for h in range(NH):
                        o_ps = psO.tile([128, TQ], F32, tag="o", name="o_ps")
                        u_ps = psU.tile([128, TQ], F32, tag="u", name="u_ps")
                        n_units = 2 * tqt + 2
                        ui = 0
                        # full-width off-diagonal key-chunk pairs (fp8 DR)
                        for i in range(2 * tqt):
                            s3 = psS.tile(
                                [128, 2, TQ], F32, tag="s", name="s_ps"
                            )
                            for ii in range(2):
                                j = 2 * i + ii
                                nc.tensor.matmul(
                                    s3[:, ii, :],
                                    lhsT=kTn[j // 4][
                                        :, (j % 4) * 128:(j % 4 + 1) * 128
                                    ],
                                    rhs=qTn[tqt][:, h, :],
                                )
                            p8 = pt_pool.tile(
                                [128, 2, TQ], F8, tag="p8", name="p8"
                            )
                            nc.scalar.activation(
                                out=p8, in_=s3,
                                func=mybir.ActivationFunctionType.Exp,
                                scale=SM_SCALE, bias=nb_t,
                            )
                            push((
                                "pair", (i, p8), o_ps, u_ps, tqt, h,
                                ui == 0, False,
                            ))
                            ui += 1
                        # diagonal chunks (bf16, causal-masked)
                        for bnum, batch in enumerate(DIAG_BATCHES):
                            bw = batch[-1][3] + batch[-1][2]
                            s_ps = psS.tile(
                                [128, 2, TQ], F32, tag="s", name="s_ps"
                            )
                            s2 = s_ps.rearrange("p a b -> p (a b)")
                            for (jj, co, w, pos) in batch:
                                nc.tensor.matmul(
                                    s2[:, pos:pos + w],
                                    lhsT=kTn[tqt][
                                        :, jj * 128:(jj + 1) * 128
                                    ],
                                    rhs=qTn[tqt][:, h, co:TQ],
                                )
                            p_sb = pt_pool.tile(
                                [128, 1024], MM_DT, tag="p", name="p_sb"
                            )
                            nc.scalar.activation(
                                out=p_sb[:, 0:bw], in_=s2[:, 0:bw],
                                func=mybir.ActivationFunctionType.Exp,
                                scale=SM_SCALE, bias=nb_t,
                            )
                            for (jj, co, w, pos) in batch:
                                # causal mask: zero p where col < row
                                # (gpsimd: otherwise-idle engine)
                                nc.gpsimd.affine_select(
                                    out=p_sb[:, pos:pos + 128],
                                    in_=p_sb[:, pos:pos + 128],
                                    pattern=[[1, 128]],
                                    channel_multiplier=-1, base=0,
                                    compare_op=mybir.AluOpType.is_ge,
                                    fill=0.0,
                                )
                            push((
                                "diag", (batch, p_sb, bnum == 0),
                                o_ps, u_ps, tqt, h,
                                ui == 0, bnum == 1,
                            ))
                            ui += 1
                for item in pend:
                    flush(item)

    nc.finalize()
    return nc


@functools.lru_cache(maxsize=1)
def _get_nc():
    return build_kernel()


def make_in_maps(x, W_qkv, W_proj, q_scale, k_scale):
    x = np.asarray(x, dtype=np.float32)
    W_qkv = np.asarray(W_qkv, dtype=np.float32)
    W_proj = np.asarray(W_proj, dtype=np.float32)
    q_scale = np.asarray(q_scale, dtype=np.float32)
    k_scale = np.asarray(k_scale, dtype=np.float32)

    import ml_dtypes

    bf16 = ml_dtypes.bfloat16
    f8 = ml_dtypes.float8_e4m3
    # rms-norm folded constants: sqrt(ssq*qsc + qbi) = (16/qs)*sqrt(mean+eps)
    ws2 = WSCALE * WSCALE
    with np.errstate(divide="ignore"):
        qsc = np.ascontiguousarray(
            (ws2 / (D * q_scale**2)).reshape(D, 1)).astype(np.float32)
        qbi = np.ascontiguousarray(
            (ws2 * EPS / q_scale**2).reshape(D, 1)).astype(np.float32)
        ksc = np.ascontiguousarray(
            (ws2 / (D * k_scale**2)).reshape(D, 1)).astype(np.float32)
        kbi = np.ascontiguousarray(
            (ws2 * EPS / k_scale**2).reshape(D, 1)).astype(np.float32)
    xT_by_batch = [
        np.clip(np.ascontiguousarray(x[b].T), -224, 224).astype(f8)
        for b in range(2)
    ]
    xT16_by_batch = [
        np.ascontiguousarray(x[b].T[:, 0:TQ]).astype(bf16) for b in range(2)
    ]
    in_maps = []
    for core in range(8):
        b, g = divmod(core, 4)
        wg_full = np.ascontiguousarray(
            np.concatenate(
                [
                    W_qkv[:, 512 * g:512 * (g + 1)],
                    W_qkv[:, 2048 + 128 * g:2048 + 128 * (g + 1)],
                    W_qkv[:, 2560 + 128 * g:2560 + 128 * (g + 1)],
                ],
                axis=1,
            )
        ) * WSCALE
        wg = np.clip(wg_full, -224, 224).astype(f8)
        wg16 = wg_full.astype(bf16)
        wp = np.ascontiguousarray(
            W_proj[512 * g:512 * (g + 1), :] / WSCALE
        ).astype(bf16)
        in_maps.append(
            {
                "xT": xT_by_batch[b], "xT16": xT16_by_batch[b],
                "wg": wg, "wg16": wg16, "wp": wp,
                "qsc": qsc, "qbi": qbi, "ksc": ksc, "kbi": kbi,
            }
        )
    return in_maps


def kernel(x, W_qkv, W_proj, q_scale, k_scale):
    nc = _get_nc()
    in_maps = make_in_maps(x, W_qkv, W_proj, q_scale, k_scale)
    res = run_bass_kernel_spmd(nc, in_maps, core_ids=list(range(8)))
    outs = [r["out"] for r in res.results]
    y0 = outs[0] + outs[1] + outs[2] + outs[3]
    y1 = outs[4] + outs[5] + outs[6] + outs[7]
    return np.stack([y0, y1], axis=0).astype(np.float32)
